# revision 7
# baseline (speedup 1.0000x reference)
"""Trainium2 Bass kernel for nn_Net_46961172415327 (3-layer GraphConv + TopK pooling GNN).

Strategy (data-parallel over graphs, 8 cores, 32 graphs/core):
 - Message aggregation is reformulated as agg = A @ x with a per-graph 256x256
   adjacency-count matrix A[src, dst] built ON DEVICE: per 128-edge block,
   src/dst one-hots are built with DVE tensor_scalar(is_equal) against an iota
   row (bf16, 4x mode) and contracted on the PE (A += oneS^T @ oneD, exact
   integer counts in fp32 PSUM).
 - TopK pooling never compacts: selected-set semantics are reproduced by
   zeroing non-selected node rows (gate = tanh(score) * mask), masking scores
   of dead nodes with -1e30 in later layers, and reusing the SAME adjacency
   for all three layers.  The final output is invariant to node ordering
   inside the selected set, so only the selected SET must match the reference.
 - Per-graph exact k-th-largest thresholds come from a batched [32,256]
   max8/match_replace peel (k/8 rounds).
 - Readout: max via free-dim reduce (zeros from dead slots provably never win),
   mean via PE ones-matmul.  Final MLP + log_softmax run batched [.,32].
"""

import functools
import numpy as np

G, N, F, E = 256, 256, 128, 4096
NC = 8
GPC = G // NC            # graphs per core
KS = (128, 64, 32)
EB = E // 128            # edge blocks per graph (32)
NEG = -1.0e30


def _build_program(gpc=GPC, n_cores=NC):
    import concourse.bacc as bacc
    import concourse.mybir as mybir
    import concourse.tile as tile
    from concourse import bass

    fp32 = mybir.dt.float32
    bf16 = mybir.dt.bfloat16
    AF = mybir.ActivationFunctionType
    OP = mybir.AluOpType
    AX = mybir.AxisListType

    nc = bacc.Bacc("TRN2", target_bir_lowering=False, debug=False,
                   num_devices=n_cores)

    # ---- DRAM tensors ----
    x_d = nc.dram_tensor("x", [gpc * N, F], fp32, kind="ExternalInput")
    src_d = nc.dram_tensor("src", [gpc, E], mybir.dt.int32, kind="ExternalInput")
    dst_d = nc.dram_tensor("dst", [gpc, E], mybir.dt.int32, kind="ExternalInput")
    wts = {}
    for l in (1, 2, 3):
        wts[f"W_root{l}"] = nc.dram_tensor(f"W_root{l}", [F, F], fp32, kind="ExternalInput")
        wts[f"W_rel{l}"] = nc.dram_tensor(f"W_rel{l}", [F, F], fp32, kind="ExternalInput")
        wts[f"b{l}"] = nc.dram_tensor(f"b{l}", [F, 1], fp32, kind="ExternalInput")
        wts[f"wn{l}"] = nc.dram_tensor(f"wn{l}", [F, 1], fp32, kind="ExternalInput")
    wl1_d = nc.dram_tensor("Wl1", [6 * F, F], fp32, kind="ExternalInput")
    bl1_d = nc.dram_tensor("bl1", [F, 1], fp32, kind="ExternalInput")
    wl2_d = nc.dram_tensor("Wl2", [F, 64], fp32, kind="ExternalInput")
    bl2_d = nc.dram_tensor("bl2", [64, 1], fp32, kind="ExternalInput")
    wl3_d = nc.dram_tensor("Wl3", [64, 10], fp32, kind="ExternalInput")
    bl3_d = nc.dram_tensor("bl3", [10, 1], fp32, kind="ExternalInput")
    iota_d = nc.dram_tensor("iota_bf", [128, N], bf16, kind="ExternalInput")
    ident_d = nc.dram_tensor("ident", [128, 128], fp32, kind="ExternalInput")
    out_d = nc.dram_tensor("out", [gpc, 10], fp32, kind="ExternalOutput")

    with tile.TileContext(nc) as tc:
        with tc.tile_pool(name="persist", bufs=1) as pp, \
             tc.tile_pool(name="work", bufs=3) as wp, \
             tc.tile_pool(name="oh", bufs=6) as ohp, \
             tc.tile_pool(name="psA", bufs=1, space="PSUM") as psA_p, \
             tc.tile_pool(name="ps256", bufs=3, space="PSUM") as ps256_p, \
             tc.tile_pool(name="psT", bufs=2, space="PSUM") as psT_p, \
             tc.tile_pool(name="psAcc", bufs=1, space="PSUM") as psAcc_p:

            # ---------- constants / weights ----------
            iota_t = pp.tile([128, N], bf16)
            ident_t = pp.tile([128, 128], fp32)
            nc.sync.dma_start(out=iota_t[:], in_=iota_d.ap())
            nc.sync.dma_start(out=ident_t[:], in_=ident_d.ap())
            w_t = {}
            for l in (1, 2, 3):
                for nm in (f"W_root{l}", f"W_rel{l}"):
                    w_t[nm] = pp.tile([F, F], fp32, name=nm, tag=nm)
                    nc.sync.dma_start(out=w_t[nm][:], in_=wts[nm].ap())
                for nm in (f"b{l}", f"wn{l}"):
                    w_t[nm] = pp.tile([F, 1], fp32, name=nm, tag=nm)
                    nc.sync.dma_start(out=w_t[nm][:], in_=wts[nm].ap())
            wl1_t = pp.tile([128, 6 * F], fp32)   # chunk j at cols [128j,128j+128)
            for j in range(6):
                nc.sync.dma_start(out=wl1_t[:, j * F:(j + 1) * F],
                                  in_=wl1_d.ap()[j * F:(j + 1) * F, :])
            bl1_t = pp.tile([F, 1], fp32)
            wl2_t = pp.tile([F, 64], fp32)
            bl2_t = pp.tile([64, 1], fp32)
            wl3_t = pp.tile([64, 10], fp32)
            bl3_t = pp.tile([10, 1], fp32)
            nc.sync.dma_start(out=bl1_t[:], in_=bl1_d.ap())
            nc.sync.dma_start(out=wl2_t[:], in_=wl2_d.ap())
            nc.sync.dma_start(out=bl2_t[:], in_=bl2_d.ap())
            nc.sync.dma_start(out=wl3_t[:], in_=wl3_d.ap())
            nc.sync.dma_start(out=bl3_t[:], in_=bl3_d.ap())
            ones_t = pp.tile([128, 1], fp32)
            nc.vector.memset(ones_t[:], 1.0)
            negbig_t = pp.tile([gpc, N], fp32)
            nc.vector.memset(negbig_t[:], NEG)

            # ---------- x load: node-major [128, (2g+c)*128 + f] ----------
            x_nm = pp.tile([128, gpc * 2 * 128], fp32)
            nc.sync.dma_start(
                out=x_nm[:].rearrange("p (b f) -> p b f", f=128),
                in_=x_d.ap().rearrange("(b p) f -> p b f", p=128))

            # ---------- src/dst load: edge-partition-major [128, gpc*EB] ----
            src_i = pp.tile([128, gpc * EB], mybir.dt.int32)
            dst_i = pp.tile([128, gpc * EB], mybir.dt.int32)
            for g in range(gpc):
                nc.sync.dma_start(
                    out=src_i[:, g * EB:(g + 1) * EB],
                    in_=src_d.ap()[g].rearrange("(b p) -> p b", p=128))
                nc.sync.dma_start(
                    out=dst_i[:, g * EB:(g + 1) * EB],
                    in_=dst_d.ap()[g].rearrange("(b p) -> p b", p=128))
            src_f = pp.tile([128, gpc * EB], fp32)
            dst_f = pp.tile([128, gpc * EB], fp32)
            nc.vector.tensor_copy(out=src_f[:], in_=src_i[:])
            nc.vector.tensor_copy(out=dst_f[:], in_=dst_i[:])

            # ---------- adjacency build (per graph, bf16 one-hots on DVE) ---
            adj = pp.tile([128, gpc * 2 * N], fp32)   # A[src,dst]; chunk c rows src in [128c,128c+128)
            for g in range(gpc):
                psA0 = psA_p.tile([128, N], fp32, space="PSUM", tag="psA0")
                psA1 = psA_p.tile([128, N], fp32, space="PSUM", tag="psA1")
                for b in range(EB):
                    ohS = ohp.tile([128, N], bf16, tag="ohS")
                    ohD = ohp.tile([128, N], bf16, tag="ohD")
                    col = g * EB + b
                    nc.vector.tensor_scalar(out=ohS[:], in0=iota_t[:],
                                            scalar1=src_f[:, col:col + 1],
                                            scalar2=None, op0=OP.is_equal)
                    nc.vector.tensor_scalar(out=ohD[:], in0=iota_t[:],
                                            scalar1=dst_f[:, col:col + 1],
                                            scalar2=None, op0=OP.is_equal)
                    nc.tensor.matmul(out=psA0[:], lhsT=ohS[:, 0:128],
                                     rhs=ohD[:], start=(b == 0), stop=(b == EB - 1))
                    nc.tensor.matmul(out=psA1[:], lhsT=ohS[:, 128:256],
                                     rhs=ohD[:], start=(b == 0), stop=(b == EB - 1))
                nc.scalar.copy(out=adj[:, g * 2 * N:g * 2 * N + N], in_=psA0[:])
                nc.scalar.copy(out=adj[:, g * 2 * N + N:(g + 1) * 2 * N], in_=psA1[:])

            # ---------- x^T (feature-major) for layer 1 ----------
            xT = pp.tile([128, gpc * N], fp32)        # graph g at cols [g*N,(g+1)*N)
            for g in range(gpc):
                for c in range(2):
                    psT = psT_p.tile([128, 128], fp32, space="PSUM", tag="psT")
                    nc.tensor.transpose(out=psT[:],
                                        in_=x_nm[:, (2 * g + c) * 128:(2 * g + c + 1) * 128],
                                        identity=ident_t[:])
                    nc.scalar.copy(out=xT[:, g * N + c * 128:g * N + (c + 1) * 128],
                                   in_=psT[:])

            # persistent per-layer state
            cur_nm = x_nm       # node-major current features (overwritten per layer)
            cur_T = xT          # feature-major current features
            scoresB = [pp.tile([gpc, N], fp32, name=f"scoresB{i}", tag=f"scoresB{i}") for i in range(3)]
            maskB = [None, None, None]
            gateNM = [pp.tile([128, 2 * gpc], fp32, name=f"gateNM{i}", tag=f"gateNM{i}") for i in range(3)]  # chunk c at cols [c*gpc,(c+1)*gpc)
            rmax_t = [pp.tile([128, gpc], fp32, name=f"rmax{i}", tag=f"rmax{i}") for i in range(3)]
            rmean_t = [pp.tile([128, gpc], fp32, name=f"rmean{i}", tag=f"rmean{i}") for i in range(3)]

            def compute_layer(l):
                """graph conv l (1-based): cur_nm/cur_T -> h (overwrites cur),
                plus score columns -> scoresB[l-1]."""
                Wr = w_t[f"W_root{l}"]; We = w_t[f"W_rel{l}"]
                bb = w_t[f"b{l}"]; wn = w_t[f"wn{l}"]
                psSc = psAcc_p.tile([128, 2 * gpc], fp32, space="PSUM", tag="psAcc")
                for g in range(gpc):
                    # agg^T = sum_c x_chunk^T-contract: lhsT = x_nm chunk, rhs = adj chunk
                    psAgg = ps256_p.tile([128, N], fp32, space="PSUM", tag="ps256")
                    for c in range(2):
                        nc.tensor.matmul(out=psAgg[:],
                                         lhsT=cur_nm[:, (2 * g + c) * 128:(2 * g + c + 1) * 128],
                                         rhs=adj[:, g * 2 * N + c * N:g * 2 * N + (c + 1) * N],
                                         start=(c == 0), stop=(c == 1))
                    aggT = wp.tile([128, N], fp32, tag="aggT")
                    nc.scalar.copy(out=aggT[:], in_=psAgg[:])
                    # hpre^T = W_rel^T agg^T + W_root^T x^T
                    psH = ps256_p.tile([128, N], fp32, space="PSUM", tag="ps256")
                    nc.tensor.matmul(out=psH[:], lhsT=We[:], rhs=aggT[:],
                                     start=True, stop=False)
                    nc.tensor.matmul(out=psH[:], lhsT=Wr[:],
                                     rhs=cur_T[:, g * N:(g + 1) * N],
                                     start=False, stop=True)
                    # h^T = relu(hpre^T + b)  (overwrite cur_T slot g)
                    nc.scalar.activation(out=cur_T[:, g * N:(g + 1) * N], in_=psH[:],
                                         func=AF.Relu, bias=bb[:], scale=1.0)
                    # score columns (node-major): psSc[:, 2g+c] = hT_chunk^T @ wn
                    for c in range(2):
                        nc.tensor.matmul(out=psSc[:, c * gpc + g:c * gpc + g + 1],
                                         lhsT=cur_T[:, g * N + c * 128:g * N + (c + 1) * 128],
                                         rhs=wn[:], start=True, stop=True)
                    # h node-major (overwrite cur_nm slots)
                    for c in range(2):
                        psT = psT_p.tile([128, 128], fp32, space="PSUM", tag="psT")
                        nc.tensor.transpose(out=psT[:],
                                            in_=cur_T[:, g * N + c * 128:g * N + (c + 1) * 128],
                                            identity=ident_t[:])
                        nc.scalar.copy(out=cur_nm[:, (2 * g + c) * 128:(2 * g + c + 1) * 128],
                                       in_=psT[:])
                # scores node-major -> batched [gpc, N]
                sNM = wp.tile([128, 2 * gpc], fp32, tag="sNM")
                nc.vector.tensor_copy(out=sNM[:], in_=psSc[:])
                for c in range(2):
                    psT2 = psT_p.tile([gpc, 128], fp32, space="PSUM", tag="psT")
                    nc.tensor.transpose(
                        out=psT2[:],
                        in_=sNM[:, c * gpc:(c + 1) * gpc],
                        identity=ident_t[:])
                    nc.scalar.copy(out=scoresB[l - 1][:, c * 128:(c + 1) * 128], in_=psT2[:])

            def topk_layer(l):
                """batched threshold selection for layer l (1-based)."""
                k = KS[l - 1]
                sB = scoresB[l - 1]
                if l > 1:
                    mI = wp.tile([gpc, N], fp32, tag="mI")
                    nc.vector.tensor_scalar(out=mI[:], in0=maskB[l - 2][:],
                                            scalar1=0.5, scalar2=None, op0=OP.is_lt)
                    nc.vector.scalar_tensor_tensor(out=sB[:], in0=mI[:], scalar=NEG,
                                                   in1=sB[:], op0=OP.mult, op1=OP.add)
                work = wp.tile([gpc, N], fp32, tag="pwork")
                nc.vector.tensor_copy(out=work[:], in_=sB[:])
                m8 = None
                for r in range(k // 8):
                    m8 = wp.tile([gpc, 8], fp32, tag="m8")
                    nc.vector.max(out=m8[:], in_=work[:])
                    if r != k // 8 - 1:
                        nc.vector.match_replace(out=work[:], in_to_replace=m8[:],
                                                in_values=work[:], imm_value=NEG)
                mB = pp.tile([gpc, N], fp32, tag=f"mask{l}")
                nc.vector.tensor_scalar(out=mB[:], in0=sB[:],
                                        scalar1=m8[:, 7:8], scalar2=None,
                                        op0=OP.is_ge)
                maskB[l - 1] = mB
                tanhB = wp.tile([gpc, N], fp32, tag="tanhB")
                nc.scalar.activation(out=tanhB[:], in_=sB[:], func=AF.Tanh)
                gB = wp.tile([gpc, N], fp32, tag="gB")
                nc.vector.tensor_tensor(out=gB[:], in0=tanhB[:], in1=mB[:],
                                        op=OP.mult)
                # transpose gate to node-major [128, 2*gpc]
                for c in range(2):
                    psT2 = psT_p.tile([128, gpc], fp32, space="PSUM", tag="psT")
                    nc.tensor.transpose(out=psT2[:],
                                        in_=gB[:, c * 128:(c + 1) * 128],
                                        identity=ident_t[:gpc, :gpc])
                    nc.scalar.copy(out=gateNM[l - 1][:, c * gpc:(c + 1) * gpc], in_=psT2[:])

            def apply_gate_and_readout(l):
                """x_{l+1} = h_l * gate_l (both layouts); readout r_l."""
                k = KS[l - 1]
                psRM = psAcc_p.tile([128, gpc], fp32, space="PSUM", tag="psAcc")
                for g in range(gpc):
                    for c in range(2):
                        nc.vector.tensor_scalar(
                            out=cur_nm[:, (2 * g + c) * 128:(2 * g + c + 1) * 128],
                            in0=cur_nm[:, (2 * g + c) * 128:(2 * g + c + 1) * 128],
                            scalar1=gateNM[l - 1][:, c * gpc + g:c * gpc + g + 1],
                            scalar2=None, op0=OP.mult)
                    # x^T via transpose of gated node-major
                    for c in range(2):
                        psT = psT_p.tile([128, 128], fp32, space="PSUM", tag="psT")
                        nc.tensor.transpose(out=psT[:],
                                            in_=cur_nm[:, (2 * g + c) * 128:(2 * g + c + 1) * 128],
                                            identity=ident_t[:])
                        nc.scalar.copy(out=cur_T[:, g * N + c * 128:g * N + (c + 1) * 128],
                                       in_=psT[:])
                    # readout: max over nodes (free dim of x^T); zeros can't win (h>=0 cases)
                    nc.vector.tensor_reduce(out=rmax_t[l - 1][:, g:g + 1],
                                            in_=cur_T[:, g * N:(g + 1) * N],
                                            axis=AX.X, op=OP.max)
                    # mean via ones-matmul: psRM[:, g] += x_chunk^T @ ones
                    for c in range(2):
                        nc.tensor.matmul(out=psRM[:, g:g + 1],
                                         lhsT=cur_nm[:, (2 * g + c) * 128:(2 * g + c + 1) * 128],
                                         rhs=ones_t[:], start=(c == 0), stop=(c == 1))
                nc.scalar.activation(out=rmean_t[l - 1][:], in_=psRM[:],
                                     func=AF.Copy, scale=1.0 / k)

            # ---------- the 3 layers ----------
            compute_layer(1)
            topk_layer(1)
            apply_gate_and_readout(1)
            compute_layer(2)
            topk_layer(2)
            apply_gate_and_readout(2)
            compute_layer(3)
            topk_layer(3)
            apply_gate_and_readout(3)

            # ---------- final MLP (batched [., gpc]) ----------
            zpieces = [rmax_t[0], rmean_t[0], rmax_t[1], rmean_t[1], rmax_t[2], rmean_t[2]]
            psZ = ps256_p.tile([128, gpc], fp32, space="PSUM", tag="ps256")
            for j in range(6):
                nc.tensor.matmul(out=psZ[:], lhsT=wl1_t[:, j * F:(j + 1) * F],
                                 rhs=zpieces[j][:], start=(j == 0), stop=(j == 5))
            z1 = wp.tile([128, gpc], fp32, tag="z1")
            nc.scalar.activation(out=z1[:], in_=psZ[:], func=AF.Relu, bias=bl1_t[:])
            psZ2 = ps256_p.tile([64, gpc], fp32, space="PSUM", tag="ps256")
            nc.tensor.matmul(out=psZ2[:], lhsT=wl2_t[:], rhs=z1[:], start=True, stop=True)
            z2 = wp.tile([64, gpc], fp32, tag="z2")
            nc.scalar.activation(out=z2[:], in_=psZ2[:], func=AF.Relu, bias=bl2_t[:])
            psZ3 = ps256_p.tile([10, gpc], fp32, space="PSUM", tag="ps256")
            nc.tensor.matmul(out=psZ3[:], lhsT=wl3_t[:], rhs=z2[:], start=True, stop=True)
            lgNM = wp.tile([10, gpc], fp32, tag="lgNM")
            nc.scalar.activation(out=lgNM[:], in_=psZ3[:], func=AF.Identity, bias=bl3_t[:])
            psL = psT_p.tile([gpc, 10], fp32, space="PSUM", tag="psT")
            nc.tensor.transpose(out=psL[:], in_=lgNM[:], identity=ident_t[:10, :10])
            lg = wp.tile([gpc, 10], fp32, tag="lg")
            nc.vector.tensor_copy(out=lg[:], in_=psL[:])
            # log-softmax along free dim
            mx = wp.tile([gpc, 1], fp32, tag="mx")
            nc.vector.tensor_reduce(out=mx[:], in_=lg[:], axis=AX.X, op=OP.max)
            nc.vector.tensor_scalar(out=lg[:], in0=lg[:], scalar1=mx[:],
                                    scalar2=None, op0=OP.subtract)
            ex = wp.tile([gpc, 10], fp32, tag="ex")
            nc.scalar.activation(out=ex[:], in_=lg[:], func=AF.Exp)
            sm = wp.tile([gpc, 1], fp32, tag="sm")
            nc.vector.tensor_reduce(out=sm[:], in_=ex[:], axis=AX.X, op=OP.add)
            lsm = wp.tile([gpc, 1], fp32, tag="lsm")
            nc.scalar.activation(out=lsm[:], in_=sm[:], func=AF.Ln)
            outt = wp.tile([gpc, 10], fp32, tag="outt")
            nc.vector.tensor_scalar(out=outt[:], in0=lg[:], scalar1=lsm[:],
                                    scalar2=None, op0=OP.subtract)
            nc.sync.dma_start(out=out_d.ap(), in_=outt[:])

    nc.compile()
    return nc


@functools.lru_cache(maxsize=2)
def _get_program(gpc=GPC, n_cores=NC):
    return _build_program(gpc, n_cores)


def make_in_maps(inputs, gpc=GPC, n_cores=NC):
    import ml_dtypes
    x = np.ascontiguousarray(np.asarray(inputs["x"], dtype=np.float32))
    src = np.ascontiguousarray(np.asarray(inputs["src"], dtype=np.int32))
    dst = np.ascontiguousarray(np.asarray(inputs["dst"], dtype=np.int32))
    shared = {}
    for l in (1, 2, 3):
        shared[f"W_root{l}"] = np.asarray(inputs[f"W_root{l}"], np.float32)
        shared[f"W_rel{l}"] = np.asarray(inputs[f"W_rel{l}"], np.float32)
        shared[f"b{l}"] = np.asarray(inputs[f"b{l}"], np.float32).reshape(F, 1)
        wpv = np.asarray(inputs[f"wp{l}"], np.float32)
        wn = (wpv / np.float32(np.sqrt(np.float64(wpv.astype(np.float64) @ wpv)))).astype(np.float32)
        shared[f"wn{l}"] = wn.reshape(F, 1)
    shared["Wl1"] = np.asarray(inputs["Wl1"], np.float32)
    shared["bl1"] = np.asarray(inputs["bl1"], np.float32).reshape(F, 1)
    shared["Wl2"] = np.asarray(inputs["Wl2"], np.float32)
    shared["bl2"] = np.asarray(inputs["bl2"], np.float32).reshape(64, 1)
    shared["Wl3"] = np.asarray(inputs["Wl3"], np.float32)
    shared["bl3"] = np.asarray(inputs["bl3"], np.float32).reshape(10, 1)
    shared["iota_bf"] = np.broadcast_to(
        np.arange(N, dtype=np.float32), (128, N)).astype(ml_dtypes.bfloat16)
    shared["ident"] = np.eye(128, dtype=np.float32)
    in_maps = []
    for c in range(n_cores):
        g0 = c * gpc
        m = dict(shared)
        m["x"] = np.ascontiguousarray(x[g0:g0 + gpc].reshape(gpc * N, F))
        m["src"] = np.ascontiguousarray(src[g0:g0 + gpc])
        m["dst"] = np.ascontiguousarray(dst[g0:g0 + gpc])
        in_maps.append(m)
    return in_maps


def kernel(**inputs):
    from concourse.bass_utils import run_bass_kernel_spmd
    nc = _get_program()
    in_maps = make_in_maps(inputs)
    res = run_bass_kernel_spmd(nc, in_maps, core_ids=list(range(NC)))
    out = np.concatenate([res.results[c]["out"] for c in range(NC)], axis=0)
    return out.astype(np.float32)


if __name__ == "__main__":
    import sys
    sys.path.insert(0, "/root/problem")
    import reference
    inputs = {k: np.asarray(v) for k, v in reference.setup_inputs().items()}
    out = kernel(**inputs)
    print("kernel out", out.shape, out.dtype)
    print(out[:2])


# revision 10
# speedup vs baseline: 768.3710x; 768.3710x over previous
"""Trainium2 Bass kernel for nn_Net_46961172415327 (3-layer GraphConv + TopK pooling GNN).

Strategy (data-parallel over graphs, 8 cores, 32 graphs/core):
 - Message aggregation is reformulated as agg = A @ x with a per-graph 256x256
   adjacency-count matrix A[src, dst] built ON DEVICE: per 128-edge block,
   src/dst one-hots are built with DVE tensor_scalar(is_equal) against an iota
   row (bf16, 4x mode) and contracted on the PE (A += oneS^T @ oneD, exact
   integer counts in fp32 PSUM).
 - TopK pooling never compacts: selected-set semantics are reproduced by
   zeroing non-selected node rows (gate = tanh(score) * mask), masking scores
   of dead nodes with -1e30 in later layers, and reusing the SAME adjacency
   for all three layers.  The final output is invariant to node ordering
   inside the selected set, so only the selected SET must match the reference.
 - Per-graph exact k-th-largest thresholds come from a batched [32,256]
   max8/match_replace peel (k/8 rounds).
 - Readout: max via free-dim reduce (zeros from dead slots provably never win),
   mean via PE ones-matmul.  Final MLP + log_softmax run batched [.,32].
"""

import functools
import numpy as np

G, N, F, E = 256, 256, 128, 4096
NC = 8
GPC = G // NC            # graphs per core
KS = (128, 64, 32)
EB = E // 128            # edge blocks per graph (32)
NEG = -1.0e30


def _build_program(gpc=GPC, n_cores=NC, repeat=1):
    import concourse.bacc as bacc
    import concourse.mybir as mybir
    import concourse.tile as tile
    from concourse import bass

    fp32 = mybir.dt.float32
    bf16 = mybir.dt.bfloat16
    AF = mybir.ActivationFunctionType
    OP = mybir.AluOpType
    AX = mybir.AxisListType

    nc = bacc.Bacc("TRN2", target_bir_lowering=False, debug=False,
                   num_devices=n_cores)

    # ---- DRAM tensors ----
    x_d = nc.dram_tensor("x", [gpc * N, F], fp32, kind="ExternalInput")
    src_d = nc.dram_tensor("src", [128, gpc * (E // 128)], mybir.dt.int32, kind="ExternalInput")
    dst_d = nc.dram_tensor("dst", [128, gpc * (E // 128)], mybir.dt.int32, kind="ExternalInput")
    wts = {}
    for l in (1, 2, 3):
        wts[f"W_root{l}"] = nc.dram_tensor(f"W_root{l}", [F, F], fp32, kind="ExternalInput")
        wts[f"W_rel{l}"] = nc.dram_tensor(f"W_rel{l}", [F, F], fp32, kind="ExternalInput")
        wts[f"b{l}"] = nc.dram_tensor(f"b{l}", [F, 1], fp32, kind="ExternalInput")
        wts[f"wn{l}"] = nc.dram_tensor(f"wn{l}", [F, 1], fp32, kind="ExternalInput")
    wl1_d = nc.dram_tensor("Wl1", [6 * F, F], fp32, kind="ExternalInput")
    bl1_d = nc.dram_tensor("bl1", [F, 1], fp32, kind="ExternalInput")
    wl2_d = nc.dram_tensor("Wl2", [F, 64], fp32, kind="ExternalInput")
    bl2_d = nc.dram_tensor("bl2", [64, 1], fp32, kind="ExternalInput")
    wl3_d = nc.dram_tensor("Wl3", [64, 10], fp32, kind="ExternalInput")
    bl3_d = nc.dram_tensor("bl3", [10, 1], fp32, kind="ExternalInput")
    iota_d = nc.dram_tensor("iota_bf", [128, N], bf16, kind="ExternalInput")
    ident_d = nc.dram_tensor("ident", [128, 128], fp32, kind="ExternalInput")
    out_d = nc.dram_tensor("out", [gpc, 10], fp32, kind="ExternalOutput")

    import contextlib
    with tile.TileContext(nc) as tc:
        rep_ctx = tc.For_i(0, repeat, 1) if repeat > 1 else contextlib.nullcontext()
        with rep_ctx, \
             tc.tile_pool(name="persist", bufs=1) as pp, \
             tc.tile_pool(name="work", bufs=3) as wp, \
             tc.tile_pool(name="oh", bufs=10) as ohp, \
             tc.tile_pool(name="psA", bufs=2, space="PSUM") as psA_p, \
             tc.tile_pool(name="ps256", bufs=2, space="PSUM") as ps256_p, \
             tc.tile_pool(name="psT", bufs=1, space="PSUM") as psT_p, \
             tc.tile_pool(name="psAcc", bufs=1, space="PSUM") as psAcc_p:

            # ---------- constants / weights ----------
            iota_t = pp.tile([128, N], bf16)
            ident_t = pp.tile([128, 128], fp32)
            nc.sync.dma_start(out=iota_t[:], in_=iota_d.ap())
            nc.sync.dma_start(out=ident_t[:], in_=ident_d.ap())
            w_t = {}
            for l in (1, 2, 3):
                for nm in (f"W_root{l}", f"W_rel{l}"):
                    w_t[nm] = pp.tile([F, F], fp32, name=nm, tag=nm)
                    nc.sync.dma_start(out=w_t[nm][:], in_=wts[nm].ap())
                for nm in (f"b{l}", f"wn{l}"):
                    w_t[nm] = pp.tile([F, 1], fp32, name=nm, tag=nm)
                    nc.sync.dma_start(out=w_t[nm][:], in_=wts[nm].ap())
            wl1_t = pp.tile([128, 6 * F], fp32)   # chunk j at cols [128j,128j+128)
            for j in range(6):
                nc.sync.dma_start(out=wl1_t[:, j * F:(j + 1) * F],
                                  in_=wl1_d.ap()[j * F:(j + 1) * F, :])
            bl1_t = pp.tile([F, 1], fp32)
            wl2_t = pp.tile([F, 64], fp32)
            bl2_t = pp.tile([64, 1], fp32)
            wl3_t = pp.tile([64, 10], fp32)
            bl3_t = pp.tile([10, 1], fp32)
            nc.sync.dma_start(out=bl1_t[:], in_=bl1_d.ap())
            nc.sync.dma_start(out=wl2_t[:], in_=wl2_d.ap())
            nc.sync.dma_start(out=bl2_t[:], in_=bl2_d.ap())
            nc.sync.dma_start(out=wl3_t[:], in_=wl3_d.ap())
            nc.sync.dma_start(out=bl3_t[:], in_=bl3_d.ap())
            ones_t = pp.tile([128, 1], fp32)
            nc.vector.memset(ones_t[:], 1.0)
            negbig_t = pp.tile([gpc, N], fp32)
            nc.vector.memset(negbig_t[:], NEG)

            # ---------- src/dst load: edge-partition-major [128, gpc*EB] ----
            src_i = pp.tile([128, gpc * EB], mybir.dt.int32)
            dst_i = pp.tile([128, gpc * EB], mybir.dt.int32)
            nc.sync.dma_start(out=src_i[:], in_=src_d.ap())
            nc.sync.dma_start(out=dst_i[:], in_=dst_d.ap())
            src_f = pp.tile([128, gpc * EB], fp32)
            dst_f = pp.tile([128, gpc * EB], fp32)
            nc.vector.tensor_copy(out=src_f[:], in_=src_i[:])
            nc.vector.tensor_copy(out=dst_f[:], in_=dst_i[:])

            # ---------- x load: node-major [128, (2g+c)*128 + f] ----------
            x_nm = pp.tile([128, gpc * 2 * 128], fp32)
            nc.sync.dma_start(
                out=x_nm[:].rearrange("p (b f) -> p b f", f=128),
                in_=x_d.ap().rearrange("(b p) f -> p b f", p=128))

            # ---------- adjacency build (per graph, bf16 one-hots on DVE) ---
            adj = pp.tile([128, gpc * 2 * N], fp32)   # A[src,dst]; chunk c rows src in [128c,128c+128)
            for g in range(gpc):
                psA0 = psA_p.tile([128, N], fp32, space="PSUM", tag="psA0")
                psA1 = psA_p.tile([128, N], fp32, space="PSUM", tag="psA1")
                for b in range(EB):
                    ohS = ohp.tile([128, N], bf16, tag="ohS")
                    ohD = ohp.tile([128, N], bf16, tag="ohD")
                    col = g * EB + b
                    nc.vector.tensor_scalar(out=ohS[:], in0=iota_t[:],
                                            scalar1=src_f[:, col:col + 1],
                                            scalar2=None, op0=OP.is_equal)
                    nc.vector.tensor_scalar(out=ohD[:], in0=iota_t[:],
                                            scalar1=dst_f[:, col:col + 1],
                                            scalar2=None, op0=OP.is_equal)
                    nc.tensor.matmul(out=psA0[:], lhsT=ohS[:, 0:128],
                                     rhs=ohD[:], start=(b == 0), stop=(b == EB - 1))
                    nc.tensor.matmul(out=psA1[:], lhsT=ohS[:, 128:256],
                                     rhs=ohD[:], start=(b == 0), stop=(b == EB - 1))
                nc.scalar.copy(out=adj[:, g * 2 * N:g * 2 * N + N], in_=psA0[:])
                nc.scalar.copy(out=adj[:, g * 2 * N + N:(g + 1) * 2 * N], in_=psA1[:])

            # ---------- x^T (feature-major) for layer 1 ----------
            xT = pp.tile([128, gpc * N], fp32)        # graph g at cols [g*N,(g+1)*N)
            for g in range(gpc):
                for c in range(2):
                    psT = psT_p.tile([128, 128], fp32, space="PSUM", tag="psT")
                    nc.tensor.transpose(out=psT[:],
                                        in_=x_nm[:, (2 * g + c) * 128:(2 * g + c + 1) * 128],
                                        identity=ident_t[:])
                    nc.scalar.copy(out=xT[:, g * N + c * 128:g * N + (c + 1) * 128],
                                   in_=psT[:])

            # persistent per-layer state
            cur_nm = x_nm       # node-major current features (overwritten per layer)
            cur_T = xT          # feature-major current features
            scoresB = [pp.tile([gpc, N], fp32, name=f"scoresB{i}", tag=f"scoresB{i}") for i in range(3)]
            maskB = [None, None, None]
            gateNM = [pp.tile([128, 2 * gpc], fp32, name=f"gateNM{i}", tag=f"gateNM{i}") for i in range(3)]  # chunk c at cols [c*gpc,(c+1)*gpc)
            rmax_t = [pp.tile([128, gpc], fp32, name=f"rmax{i}", tag=f"rmax{i}") for i in range(3)]
            rmean_t = [pp.tile([128, gpc], fp32, name=f"rmean{i}", tag=f"rmean{i}") for i in range(3)]

            def compute_layer(l):
                """graph conv l (1-based): cur_nm/cur_T -> h (overwrites cur),
                plus score columns -> scoresB[l-1]."""
                Wr = w_t[f"W_root{l}"]; We = w_t[f"W_rel{l}"]
                bb = w_t[f"b{l}"]; wn = w_t[f"wn{l}"]
                psSc = psAcc_p.tile([128, 2 * gpc], fp32, space="PSUM", tag="psAcc")
                for g in range(gpc):
                    # agg^T = sum_c x_chunk^T-contract: lhsT = x_nm chunk, rhs = adj chunk
                    psAgg = ps256_p.tile([128, N], fp32, space="PSUM", tag="ps256")
                    for c in range(2):
                        nc.tensor.matmul(out=psAgg[:],
                                         lhsT=cur_nm[:, (2 * g + c) * 128:(2 * g + c + 1) * 128],
                                         rhs=adj[:, g * 2 * N + c * N:g * 2 * N + (c + 1) * N],
                                         start=(c == 0), stop=(c == 1))
                    aggT = wp.tile([128, N], fp32, tag="aggT")
                    nc.scalar.copy(out=aggT[:], in_=psAgg[:])
                    # hpre^T = W_rel^T agg^T + W_root^T x^T
                    psH = ps256_p.tile([128, N], fp32, space="PSUM", tag="ps256")
                    nc.tensor.matmul(out=psH[:], lhsT=We[:], rhs=aggT[:],
                                     start=True, stop=False)
                    nc.tensor.matmul(out=psH[:], lhsT=Wr[:],
                                     rhs=cur_T[:, g * N:(g + 1) * N],
                                     start=False, stop=True)
                    # h^T = relu(hpre^T + b)  (overwrite cur_T slot g)
                    nc.scalar.activation(out=cur_T[:, g * N:(g + 1) * N], in_=psH[:],
                                         func=AF.Relu, bias=bb[:], scale=1.0)
                    # score columns (node-major): psSc[:, 2g+c] = hT_chunk^T @ wn
                    for c in range(2):
                        nc.tensor.matmul(out=psSc[:, c * gpc + g:c * gpc + g + 1],
                                         lhsT=cur_T[:, g * N + c * 128:g * N + (c + 1) * 128],
                                         rhs=wn[:], start=True, stop=True)
                    # h node-major (overwrite cur_nm slots)
                    for c in range(2):
                        psT = psT_p.tile([128, 128], fp32, space="PSUM", tag="psT")
                        nc.tensor.transpose(out=psT[:],
                                            in_=cur_T[:, g * N + c * 128:g * N + (c + 1) * 128],
                                            identity=ident_t[:])
                        nc.scalar.copy(out=cur_nm[:, (2 * g + c) * 128:(2 * g + c + 1) * 128],
                                       in_=psT[:])
                # scores node-major -> batched [gpc, N]
                sNM = wp.tile([128, 2 * gpc], fp32, tag="sNM")
                nc.vector.tensor_copy(out=sNM[:], in_=psSc[:])
                for c in range(2):
                    psT2 = psT_p.tile([gpc, 128], fp32, space="PSUM", tag="psT")
                    nc.tensor.transpose(
                        out=psT2[:],
                        in_=sNM[:, c * gpc:(c + 1) * gpc],
                        identity=ident_t[:])
                    nc.scalar.copy(out=scoresB[l - 1][:, c * 128:(c + 1) * 128], in_=psT2[:])

            def topk_layer(l):
                """batched threshold selection for layer l (1-based)."""
                k = KS[l - 1]
                sB = scoresB[l - 1]
                if l > 1:
                    mI = wp.tile([gpc, N], fp32, tag="mI")
                    nc.vector.tensor_scalar(out=mI[:], in0=maskB[l - 2][:],
                                            scalar1=0.5, scalar2=None, op0=OP.is_lt)
                    nc.vector.scalar_tensor_tensor(out=sB[:], in0=mI[:], scalar=NEG,
                                                   in1=sB[:], op0=OP.mult, op1=OP.add)
                work = wp.tile([gpc, N], fp32, tag="pwork")
                nc.vector.tensor_copy(out=work[:], in_=sB[:])
                m8 = None
                for r in range(k // 8):
                    m8 = wp.tile([gpc, 8], fp32, tag="m8")
                    nc.vector.max(out=m8[:], in_=work[:])
                    if r != k // 8 - 1:
                        nc.vector.match_replace(out=work[:], in_to_replace=m8[:],
                                                in_values=work[:], imm_value=NEG)
                mB = pp.tile([gpc, N], fp32, tag=f"mask{l}")
                nc.vector.tensor_scalar(out=mB[:], in0=sB[:],
                                        scalar1=m8[:, 7:8], scalar2=None,
                                        op0=OP.is_ge)
                maskB[l - 1] = mB
                tanhB = wp.tile([gpc, N], fp32, tag="tanhB")
                nc.scalar.activation(out=tanhB[:], in_=sB[:], func=AF.Tanh)
                gB = wp.tile([gpc, N], fp32, tag="gB")
                nc.vector.tensor_tensor(out=gB[:], in0=tanhB[:], in1=mB[:],
                                        op=OP.mult)
                # transpose gate to node-major [128, 2*gpc]
                for c in range(2):
                    psT2 = psT_p.tile([128, gpc], fp32, space="PSUM", tag="psT")
                    nc.tensor.transpose(out=psT2[:],
                                        in_=gB[:, c * 128:(c + 1) * 128],
                                        identity=ident_t[:gpc, :gpc])
                    nc.scalar.copy(out=gateNM[l - 1][:, c * gpc:(c + 1) * gpc], in_=psT2[:])

            def apply_gate_and_readout(l):
                """x_{l+1} = h_l * gate_l (both layouts); readout r_l."""
                k = KS[l - 1]
                psRM = psAcc_p.tile([128, gpc], fp32, space="PSUM", tag="psAcc")
                for g in range(gpc):
                    for c in range(2):
                        nc.vector.tensor_scalar(
                            out=cur_nm[:, (2 * g + c) * 128:(2 * g + c + 1) * 128],
                            in0=cur_nm[:, (2 * g + c) * 128:(2 * g + c + 1) * 128],
                            scalar1=gateNM[l - 1][:, c * gpc + g:c * gpc + g + 1],
                            scalar2=None, op0=OP.mult)
                    # x^T via transpose of gated node-major
                    for c in range(2):
                        psT = psT_p.tile([128, 128], fp32, space="PSUM", tag="psT")
                        nc.tensor.transpose(out=psT[:],
                                            in_=cur_nm[:, (2 * g + c) * 128:(2 * g + c + 1) * 128],
                                            identity=ident_t[:])
                        nc.scalar.copy(out=cur_T[:, g * N + c * 128:g * N + (c + 1) * 128],
                                       in_=psT[:])
                    # readout: max over nodes (free dim of x^T); zeros can't win (h>=0 cases)
                    nc.vector.tensor_reduce(out=rmax_t[l - 1][:, g:g + 1],
                                            in_=cur_T[:, g * N:(g + 1) * N],
                                            axis=AX.X, op=OP.max)
                    # mean via ones-matmul: psRM[:, g] += x_chunk^T @ ones
                    for c in range(2):
                        nc.tensor.matmul(out=psRM[:, g:g + 1],
                                         lhsT=cur_nm[:, (2 * g + c) * 128:(2 * g + c + 1) * 128],
                                         rhs=ones_t[:], start=(c == 0), stop=(c == 1))
                nc.scalar.activation(out=rmean_t[l - 1][:], in_=psRM[:],
                                     func=AF.Copy, scale=1.0 / k)

            # ---------- the 3 layers ----------
            compute_layer(1)
            topk_layer(1)
            apply_gate_and_readout(1)
            compute_layer(2)
            topk_layer(2)
            apply_gate_and_readout(2)
            compute_layer(3)
            topk_layer(3)
            apply_gate_and_readout(3)

            # ---------- final MLP (batched [., gpc]) ----------
            zpieces = [rmax_t[0], rmean_t[0], rmax_t[1], rmean_t[1], rmax_t[2], rmean_t[2]]
            psZ = ps256_p.tile([128, gpc], fp32, space="PSUM", tag="ps256")
            for j in range(6):
                nc.tensor.matmul(out=psZ[:], lhsT=wl1_t[:, j * F:(j + 1) * F],
                                 rhs=zpieces[j][:], start=(j == 0), stop=(j == 5))
            z1 = wp.tile([128, gpc], fp32, tag="z1")
            nc.scalar.activation(out=z1[:], in_=psZ[:], func=AF.Relu, bias=bl1_t[:])
            psZ2 = ps256_p.tile([64, gpc], fp32, space="PSUM", tag="ps256")
            nc.tensor.matmul(out=psZ2[:], lhsT=wl2_t[:], rhs=z1[:], start=True, stop=True)
            z2 = wp.tile([64, gpc], fp32, tag="z2")
            nc.scalar.activation(out=z2[:], in_=psZ2[:], func=AF.Relu, bias=bl2_t[:])
            psZ3 = ps256_p.tile([10, gpc], fp32, space="PSUM", tag="ps256")
            nc.tensor.matmul(out=psZ3[:], lhsT=wl3_t[:], rhs=z2[:], start=True, stop=True)
            lgNM = wp.tile([10, gpc], fp32, tag="lgNM")
            nc.scalar.activation(out=lgNM[:], in_=psZ3[:], func=AF.Identity, bias=bl3_t[:])
            psL = psT_p.tile([gpc, 10], fp32, space="PSUM", tag="psT")
            nc.tensor.transpose(out=psL[:], in_=lgNM[:], identity=ident_t[:10, :10])
            lg = wp.tile([gpc, 10], fp32, tag="lg")
            nc.vector.tensor_copy(out=lg[:], in_=psL[:])
            # log-softmax along free dim
            mx = wp.tile([gpc, 1], fp32, tag="mx")
            nc.vector.tensor_reduce(out=mx[:], in_=lg[:], axis=AX.X, op=OP.max)
            nc.vector.tensor_scalar(out=lg[:], in0=lg[:], scalar1=mx[:],
                                    scalar2=None, op0=OP.subtract)
            ex = wp.tile([gpc, 10], fp32, tag="ex")
            nc.scalar.activation(out=ex[:], in_=lg[:], func=AF.Exp)
            sm = wp.tile([gpc, 1], fp32, tag="sm")
            nc.vector.tensor_reduce(out=sm[:], in_=ex[:], axis=AX.X, op=OP.add)
            lsm = wp.tile([gpc, 1], fp32, tag="lsm")
            nc.scalar.activation(out=lsm[:], in_=sm[:], func=AF.Ln)
            outt = wp.tile([gpc, 10], fp32, tag="outt")
            nc.vector.tensor_scalar(out=outt[:], in0=lg[:], scalar1=lsm[:],
                                    scalar2=None, op0=OP.subtract)
            nc.sync.dma_start(out=out_d.ap(), in_=outt[:])

    nc.compile()
    return nc


@functools.lru_cache(maxsize=2)
def _get_program(gpc=GPC, n_cores=NC):
    return _build_program(gpc, n_cores)



def make_in_maps(inputs, gpc=GPC, n_cores=NC):
    import ml_dtypes
    x = np.ascontiguousarray(np.asarray(inputs["x"], dtype=np.float32))
    src = np.ascontiguousarray(np.asarray(inputs["src"], dtype=np.int32))
    dst = np.ascontiguousarray(np.asarray(inputs["dst"], dtype=np.int32))
    shared = {}
    for l in (1, 2, 3):
        shared[f"W_root{l}"] = np.asarray(inputs[f"W_root{l}"], np.float32)
        shared[f"W_rel{l}"] = np.asarray(inputs[f"W_rel{l}"], np.float32)
        shared[f"b{l}"] = np.asarray(inputs[f"b{l}"], np.float32).reshape(F, 1)
        wpv = np.asarray(inputs[f"wp{l}"], np.float32)
        wn = (wpv / np.float32(np.sqrt(np.float64(wpv.astype(np.float64) @ wpv)))).astype(np.float32)
        shared[f"wn{l}"] = wn.reshape(F, 1)
    shared["Wl1"] = np.asarray(inputs["Wl1"], np.float32)
    shared["bl1"] = np.asarray(inputs["bl1"], np.float32).reshape(F, 1)
    shared["Wl2"] = np.asarray(inputs["Wl2"], np.float32)
    shared["bl2"] = np.asarray(inputs["bl2"], np.float32).reshape(64, 1)
    shared["Wl3"] = np.asarray(inputs["Wl3"], np.float32)
    shared["bl3"] = np.asarray(inputs["bl3"], np.float32).reshape(10, 1)
    shared["iota_bf"] = np.broadcast_to(
        np.arange(N, dtype=np.float32), (128, N)).astype(ml_dtypes.bfloat16)
    shared["ident"] = np.eye(128, dtype=np.float32)
    in_maps = []
    for c in range(n_cores):
        g0 = c * gpc
        m = dict(shared)
        m["x"] = np.ascontiguousarray(x[g0:g0 + gpc].reshape(gpc * N, F))
        # edge-partition-major: srcT[p, g*EB+b] = src[g, 128*b+p]
        m["src"] = np.ascontiguousarray(
            src[g0:g0 + gpc].reshape(gpc, E // 128, 128).transpose(2, 0, 1).reshape(128, -1))
        m["dst"] = np.ascontiguousarray(
            dst[g0:g0 + gpc].reshape(gpc, E // 128, 128).transpose(2, 0, 1).reshape(128, -1))
        in_maps.append(m)
    return in_maps


def kernel(**inputs):
    from concourse.bass_utils import run_bass_kernel_spmd
    nc = _get_program()
    in_maps = make_in_maps(inputs)
    res = run_bass_kernel_spmd(nc, in_maps, core_ids=list(range(NC)))
    out = np.concatenate([res.results[c]["out"] for c in range(NC)], axis=0)
    return out.astype(np.float32)


if __name__ == "__main__":
    import sys
    sys.path.insert(0, "/root/problem")
    import reference
    inputs = {k: np.asarray(v) for k, v in reference.setup_inputs().items()}
    out = kernel(**inputs)
    print("kernel out", out.shape, out.dtype)
    print(out[:2])
